# revision 15
# baseline (speedup 1.0000x reference)
"""Single-head causal attention on 8 TRN2 NeuronCores.

Problem shapes (hardcoded): B=8, T=2048, C=1024, H=64, fp32 I/O.
    q = x @ Wq; k = x @ Wk; v = x @ Wv          (per batch element)
    wei = softmax(causal_mask(q @ k.T * C**-0.5))
    out = wei @ v
Sharding: pure data parallel - one batch element per core, no collectives.

Per-core algorithm (fp8 DoubleRow projections, bf16 attention, fp32 PSUM):
  - host quantizes xT to e4m3 (x8) plus an e4m3 residual r8 = xT - x8, and
    weights to e4m3 at folded scales (Wqk*16, Wv*32) with e4m3 weight
    residuals.  QKV projections run as fp8 DoubleRow matmuls (2 C-rows per
    partition element, 2 output cols per cycle):
      qk' = x8 @ (Wqk8 + Wqkr8)               (= 16*[q|k] + O(0.03%))
      v'  = x8 @ (Wv8 + Wvr8) + r8 @ Wv8      (= 32*v    + O(0.2%))
    measured end-to-end rel err 0.97% (vs 0.41% all-bf16, 2% tolerance);
    the fp8 input stream is 2x2 MB, matching bf16 bytes but half the PE
    cycles for the projections.
  - S^T row-packed: kT2 holds Tk pairs in partition halves, qT2hi holds q
    duplicated into the hi half; h0 reads q straight from the projection
    cast (qkT rows 0:64).  All half-shuffles are partition-shifted engine
    copies (no DMA): Pool shifts SBUF->SBUF, DVE drains PSUM.  exp scale
    folds the 16x: P = exp(S'/8192).  Causal diag blocks masked on Pool.
  - v1 = [32*v | 32.0] so [num|den] share one accumulator and the 32x
    cancels in the normalize (rc = 1/(32 den), ot = 32 num * rc).
  - HW-DGE DMA queues (sync/scalar) carry ONLY input streams + output
    stores, ~16 instructions total: the queues share 8 completion
    semaphores and every extra DMA builds false cross-queue serialization
    chains.  Outputs collect in one SBUF buffer; slices 0-2 store as one
    DMA, slice 3 per-block to keep the tail short.
  - per-slice pipeline: fp8 projections -> casts/shifts -> S^T pair tiles
    feed ScalarE exp (~17us, the secondary bottleneck; PE ~19us);
    AV trails exp by one tile; v-finish waits for r8; epilogue deferred
    one slice.
"""

import numpy as np
import ml_dtypes

import concourse.bass as bass
import concourse.mybir as mybir
import concourse.tile as tile
from concourse import bacc
from concourse.bass_utils import run_bass_kernel_spmd

B, T, C, H = 8, 2048, 1024, 64
NT = T // 128           # 16 Tk-blocks of 128
NJ = T // 512           # 4 Tq-slices of 512
SCALE = (float(C) ** -0.5) / 256.0  # exp scale with 16x-weight fold

BF16 = mybir.dt.bfloat16
F32 = mybir.dt.float32
F8 = mybir.dt.float8e4
DR = mybir.MatmulPerfMode.DoubleRow
npbf16 = ml_dtypes.bfloat16
npf8 = ml_dtypes.float8_e4m3fn


def build_attention(nc: bass.Bass, tc: tile.TileContext, ctx):
    x8_d = nc.dram_tensor("x8", [128, 8, T], F8, kind="ExternalInput").ap()
    r8_d = nc.dram_tensor("r8", [128, 8, T], F8, kind="ExternalInput").ap()
    # wqk[0]=Wqk8, wqk[1]=Wqkr8; wvv[0]=Wv8, wvv[1]=Wvr8
    wqk_d = nc.dram_tensor("wqk", [128, 2, 4, 2, 128], F8,
                           kind="ExternalInput").ap()
    wvv_d = nc.dram_tensor("wvv", [128, 2, 4, 2, H], F8,
                           kind="ExternalInput").ap()
    # idents: cols 0:64 = [I64;I64], 64:192 = causal, 192:257 = I65 pad
    ident_d = nc.dram_tensor("idents", [128, 257], BF16,
                             kind="ExternalInput").ap()
    out_d = nc.dram_tensor("out", [T, H], F32, kind="ExternalOutput").ap()

    consts = ctx.enter_context(tc.tile_pool(name="consts", bufs=1))
    persist = ctx.enter_context(tc.tile_pool(name="persist", bufs=1))
    pts = ctx.enter_context(tc.tile_pool(name="pts", bufs=6))
    outts = ctx.enter_context(tc.tile_pool(name="outts", bufs=2))
    smalls = ctx.enter_context(tc.tile_pool(name="smalls", bufs=2))
    ps_big = ctx.enter_context(tc.tile_pool(name="ps_big", bufs=2,
                                            space="PSUM"))
    ps_v = ctx.enter_context(tc.tile_pool(name="ps_v", bufs=2, space="PSUM"))
    ps_mix = ctx.enter_context(tc.tile_pool(name="ps_mix", bufs=2,
                                            space="PSUM"))

    # ---- input DMAs: minimal count, time-ordered across the two HW DGE
    # queues so the 8-semaphore round-robin reuse chains stay satisfied.
    wqk_sb = consts.tile([128, 2, 4, 2, 128], F8, tag="wqk")
    nc.scalar.dma_start(out=wqk_sb, in_=wqk_d)
    x8_sb = persist.tile([128, 8, T], F8, tag="x8")
    r8_sb = persist.tile([128, 8, T], F8, tag="r8")
    half = T // 2
    for h0 in (0, half):
        hs = slice(h0, h0 + half)
        nc.sync.dma_start(out=x8_sb[:, 0:4, hs], in_=x8_d[:, 0:4, hs])
        nc.scalar.dma_start(out=x8_sb[:, 4:8, hs], in_=x8_d[:, 4:8, hs])
    wvv_sb = consts.tile([128, 2, 4, 2, H], F8, tag="wvv")
    nc.sync.dma_start(out=wvv_sb, in_=wvv_d)
    ident_sb = consts.tile([128, 257], BF16, tag="idents")
    nc.scalar.dma_start(out=ident_sb, in_=ident_d)
    for h0 in (0, half):
        hs = slice(h0, h0 + half)
        nc.sync.dma_start(out=r8_sb[:, 0:4, hs], in_=r8_d[:, 0:4, hs])
        nc.scalar.dma_start(out=r8_sb[:, 4:8, hs], in_=r8_d[:, 4:8, hs])

    i64_sb = ident_sb[:, 0:64]
    causal_sb = ident_sb[:, 64:192]
    i65_sb = ident_sb[0:65, 192:257]

    qkT = persist.tile([128, T], BF16, tag="qkT")      # [16q; 16k]
    qT2hi = persist.tile([128, T], BF16, tag="qT2hi")  # 16q in rows 64:128
    kT2 = persist.tile([128, T // 2], BF16, tag="kT2")  # Tk pairs in halves
    vT = persist.tile([64, T], BF16, tag="vT")          # 32*v^T
    vT2 = persist.tile([128, T // 2], BF16, tag="vT2")  # odd Tk blocks, hi
    v1 = persist.tile([128, NT, H + 1], BF16, tag="v1")  # [32v | 32]
    nc.vector.memset(v1, 32.0)
    outbuf = persist.tile([128, NT, H], F32, tag="outbuf")

    pending_av = None
    for j in range(NJ):
        jsl = slice(j * 512, (j + 1) * 512)

        # ---- fp8 DoubleRow projections for slice j --------------------
        qk_ps = ps_mix.tile([128, 512], F32, tag="mix", name=f"qk_ps{j}")
        for r in range(2):
            for b in range(4):
                nc.tensor.matmul(qk_ps, lhsT=wqk_sb[:, r, b, :, :],
                                 rhs=x8_sb[:, 2 * b:2 * b + 2, jsl],
                                 start=(r == 0 and b == 0),
                                 stop=(r == 1 and b == 3), perf_mode=DR)
        nc.vector.tensor_copy(qkT[:, jsl], qk_ps)
        # odd k-blocks (4j+1, 4j+3) straight from PSUM into kT2 hi half
        for b in (1, 3):
            c0 = (2 * j + b // 2) * 128
            nc.vector.tensor_copy(kT2[64:128, c0:c0 + 128],
                                  qk_ps[64:128, b * 128:(b + 1) * 128])
        # partition-shifted SBUF copies on Pool: q dup, even k-blocks
        nc.gpsimd.tensor_copy(qT2hi[64:128, jsl], qkT[0:64, jsl])
        for b in (0, 2):
            c0 = (2 * j + b // 2) * 128
            nc.gpsimd.tensor_copy(
                kT2[0:64, c0:c0 + 128],
                qkT[64:128, j * 512 + b * 128:j * 512 + (b + 1) * 128])

        v_ps = ps_v.tile([128, 512], F32, tag="vps", name=f"v_ps{j}")
        for r in range(2):
            for b in range(4):
                nc.tensor.matmul(v_ps[0:64, :], lhsT=wvv_sb[:, r, b, :, :],
                                 rhs=x8_sb[:, 2 * b:2 * b + 2, jsl],
                                 start=(r == 0 and b == 0), stop=False,
                                 perf_mode=DR)
        # r8 @ Wv8 term deferred to emit_vfinish (r8 lands later)

        # ---- deferred epilogue of slice j-1 ---------------------------
        if pending_av is not None:
            emit_epilogue(nc, outts, smalls, ps_mix, i65_sb, outbuf,
                          *pending_av)
            pending_av = None
        if j == 3:  # slices 0-2 done: one bulk store for t-blocks 0:12
            nc.sync.dma_start(
                out=out_d[0:12 * 128, :].rearrange("(t p) h -> p t h", p=128),
                in_=outbuf[:, 0:12, :])

        # ---- v-finish for slice j (r8 term, cast, transpose into v1) --
        emit_vfinish(nc, ps_mix, wvv_sb, r8_sb, v_ps, vT, vT2, v1, i64_sb, j)

        # ---- attention for slice j (row-packed S^T, pipelined AV) -----
        av = ps_mix.tile([65, 512], F32, tag="mix", name=f"av{j}")
        nblk = 4 * j + 4
        prev = None
        for m in range(2 * j + 2):
            sp2 = ps_big.tile([128, 1024], F32, tag="big", name=f"sp{j}_{m}")
            pt2 = pts.tile([128, 1024], BF16, tag="pt", name=f"pt{j}_{m}")
            n0s = []
            for half_idx, i in ((0, 2 * m), (1, 2 * m + 1)):
                g = i - 4 * j
                n0 = max(0, g) * 128
                p0 = half_idx * 64
                o = half_idx * 512
                rhs = (qkT if half_idx == 0 else qT2hi)
                nc.tensor.matmul(
                    sp2[:, o + n0:o + 512],
                    lhsT=kT2[p0:p0 + 64, m * 128:(m + 1) * 128],
                    rhs=rhs[p0:p0 + 64, j * 512 + n0:(j + 1) * 512],
                    start=True, stop=True)
                n0s.append(n0)
            if n0s[0] == 0 and n0s[1] == 0:  # one wide exp over both banks
                nc.scalar.activation(pt2, sp2,
                                     mybir.ActivationFunctionType.Exp,
                                     scale=SCALE)
            else:
                for half_idx in range(2):
                    o, n0 = half_idx * 512, n0s[half_idx]
                    nc.scalar.activation(
                        pt2[:, o + n0:o + 512], sp2[:, o + n0:o + 512],
                        mybir.ActivationFunctionType.Exp, scale=SCALE)
            for half_idx, i in ((0, 2 * m), (1, 2 * m + 1)):
                g = i - 4 * j
                if g >= 0:  # mask upper triangle of the diagonal block
                    o = half_idx * 512 + n0s[half_idx]
                    nc.gpsimd.tensor_mul(
                        pt2[:, o:o + 128], pt2[:, o:o + 128], causal_sb)
            if prev is not None:
                emit_av(nc, av, v1, *prev, nblk)
            prev = (pt2, n0s, 2 * m)
        emit_av(nc, av, v1, *prev, nblk)
        pending_av = (av, j)

    emit_epilogue(nc, outts, smalls, ps_mix, i65_sb, outbuf, *pending_av,
                  out_d=out_d)


def emit_vfinish(nc, ps_mix, wvv_sb, r8_sb, v_ps, vT, vT2, v1, i64_sb, j):
    """r8 @ Wv8 correction, 32v^T cast, and transpose back into v1."""
    jsl = slice(j * 512, (j + 1) * 512)
    for b in range(4):
        nc.tensor.matmul(v_ps[0:64, :], lhsT=wvv_sb[:, 0, b, :, :],
                         rhs=r8_sb[:, 2 * b:2 * b + 2, jsl],
                         start=False, stop=(b == 3), perf_mode=DR)
    nc.vector.tensor_copy(vT[:, jsl], v_ps[0:64, :])
    # odd Tk blocks 4j+1, 4j+3 -> vT2 hi half (partition-shifted on Pool)
    for bb in range(2):
        tb = 4 * j + 2 * bb + 1
        c0 = (2 * j + bb) * 128
        nc.gpsimd.tensor_copy(vT2[64:128, c0:c0 + 128],
                              vT[:, tb * 128:(tb + 1) * 128])
    # v natural via row-packed identity matmuls (pair of Tk blocks)
    for mt in (2 * j, 2 * j + 1):
        tA, tB = 2 * mt, 2 * mt + 1
        vpA = ps_mix.tile([128, H], F32, tag="mix", name=f"vpA{mt}")
        vpB = ps_mix.tile([128, H], F32, tag="mix", name=f"vpB{mt}")
        nc.tensor.matmul(vpA, lhsT=vT[:, tA * 128:(tA + 1) * 128],
                         rhs=i64_sb[0:64, :], start=True, stop=True)
        nc.tensor.matmul(vpB, lhsT=vT2[64:128, mt * 128:(mt + 1) * 128],
                         rhs=i64_sb[64:128, :], start=True, stop=True)
        nc.vector.tensor_copy(v1[:, tA, 0:H], vpA)
        nc.vector.tensor_copy(v1[:, tB, 0:H], vpB)


def emit_av(nc, av, v1, pt2, n0s, i0, nblk):
    for d in range(2):
        i = i0 + d
        o, n0 = d * 512, n0s[d]
        nc.tensor.matmul(av[:, n0:512], lhsT=v1[:, i, :],
                         rhs=pt2[:, o + n0:o + 512],
                         start=(i == 0), stop=(i == nblk - 1))


def emit_epilogue(nc, outts, smalls, ps_mix, i65_sb, outbuf, av, j,
                  out_d=None):
    osb = outts.tile([65, 512], BF16, tag="osb", name=f"osb{j}")
    nc.vector.tensor_copy(osb, av)  # f32 PSUM -> bf16 SBUF
    for t in range(4):
        op = ps_mix.tile([128, H + 1], F32, tag="mix", name=f"op{j}_{t}")
        nc.tensor.matmul(op, lhsT=osb[:, t * 128:(t + 1) * 128], rhs=i65_sb,
                         start=True, stop=True)
        rc = smalls.tile([128, 1], F32, tag="rc", name=f"rc{j}_{t}")
        nc.vector.reciprocal(rc, op[:, H:H + 1])  # = 1/(32 den)
        tb = j * 4 + t
        nc.vector.tensor_scalar_mul(outbuf[:, tb, :], op[:, 0:H], rc)
        if out_d is not None:  # last slice: store per block for a short tail
            nc.sync.dma_start(out=out_d[tb * 128:(tb + 1) * 128, :],
                              in_=outbuf[:, tb, :])


_CACHED = {}


def _get_nc():
    if "nc" not in _CACHED:
        from contextlib import ExitStack
        nc = bacc.Bacc("TRN2", target_bir_lowering=False, debug=False,
                       num_devices=B)
        with tile.TileContext(nc) as tc:
            with ExitStack() as ctx:
                build_attention(nc, tc, ctx)
        nc.compile()
        _CACHED["nc"] = nc
    return _CACHED["nc"]


def _quant_inputs(inputs, Wq, Wk, Wv):
    """Host-side fp8 prep: per-batch x8/r8 in [128, 8, T] layout, weights
    at folded scales with e4m3 residuals stacked in [128, 2, 4, 2, M]."""
    inputs = np.asarray(inputs, dtype=np.float32)

    def wstack(w, m):  # [C, m] scaled -> [128, 2, 4, 2, m] (w8, residual8)
        w8 = w.astype(npf8)
        wr8 = (w - w8.astype(np.float32)).astype(npf8)
        pair = np.stack([w8, wr8])  # [2, C, m]
        return np.ascontiguousarray(
            pair.reshape(2, 4, 2, 128, m).transpose(3, 0, 1, 2, 4))

    wqk = np.concatenate([np.asarray(Wq), np.asarray(Wk)], axis=1)
    wqk = wstack(wqk.astype(np.float32) * 16.0, 128)
    wvv = wstack(np.asarray(Wv).astype(np.float32) * 32.0, H)

    idents = np.zeros((128, 257), dtype=npbf16)
    idents[0:64, 0:64] = np.eye(64, dtype=npbf16)
    idents[64:128, 0:64] = np.eye(64, dtype=npbf16)
    idents[:, 64:192] = np.triu(np.ones((128, 128), dtype=npbf16))
    idents[0:65, 192:257] = np.eye(65, dtype=npbf16)

    in_maps = []
    for b in range(B):
        xT = inputs[b].T  # [C, T] fp32
        x8 = xT.astype(npf8)
        r8 = (xT - x8.astype(np.float32)).astype(npf8)
        x8 = np.ascontiguousarray(x8.reshape(8, 128, T).transpose(1, 0, 2))
        r8 = np.ascontiguousarray(r8.reshape(8, 128, T).transpose(1, 0, 2))
        in_maps.append({"x8": x8, "r8": r8, "wqk": wqk, "wvv": wvv,
                        "idents": idents})
    return in_maps


def kernel(inputs, Wq, Wk, Wv):
    in_maps = _quant_inputs(inputs, Wq, Wk, Wv)
    nc = _get_nc()
    res = run_bass_kernel_spmd(nc, in_maps, core_ids=list(range(B)))
    out = np.stack([res.results[b]["out"] for b in range(B)], axis=0)
    return out.astype(np.float32)


# revision 17
# speedup vs baseline: 1.2197x; 1.2197x over previous
"""Single-head causal attention on 8 TRN2 NeuronCores.

Problem shapes (hardcoded): B=8, T=2048, C=1024, H=64, fp32 I/O.
    q = x @ Wq; k = x @ Wk; v = x @ Wv          (per batch element)
    wei = softmax(causal_mask(q @ k.T * C**-0.5))
    out = wei @ v
Sharding: pure data parallel - one batch element per core, no collectives.

Per-core algorithm (bf16 matmuls, fp32 PSUM accumulation):
  - host pre-transposes x -> xT [C, T] (C on partitions) and packs
    [Wq|Wk]; projections run per 512-wide T-slice: qkT = [Wq|Wk].T @ xT,
    vT = Wv.T @ xT.  (fp8 DoubleRow was tried and measured SLOWER: each
    matmul pays its own LDWEIGHTS, and a 256-col DR weight load outruns
    the halved 107ns stream time, 259ns/MM vs bf16's 216ns/MM.)
  - S^T row-packed: kT2 holds Tk-block pairs in the two partition halves,
    qT2hi duplicates q into the hi half; h0 reads q straight from the
    projection cast (qkT rows 0:64).  All half-shuffles are
    partition-shifted ENGINE copies, not DMAs: Pool shifts SBUF->SBUF,
    DVE drains PSUM.  P = exp(S/32), no max-subtraction (logits std
    ~0.25); causal diag blocks masked by a 0/1 multiply on Pool.
  - v1 = [v | 1] so [num|den] share one accumulator; v natural recovered
    from vT by row-packed identity matmuls.
  - HW-DGE DMA queues (sync/scalar) carry ONLY input streams + output
    stores (~16 instructions): the two queues share 8 completion
    semaphores, and extra DMAs build false cross-queue serialization
    chains.  xT streams in T-quarter x C-half chunks so slice j's
    projections unblock at ~2.7(j+1)us.  Outputs collect in one SBUF
    buffer; slices 0-2 store as one DMA, slice 3 per-block (short tail).
  - per-slice pipeline: QK proj -> casts/shifts -> S pair tiles 0,1
    hoisted BEFORE the V projection so ScalarE exp (the ~17us secondary
    bottleneck; PE ~25us) starts early and stays fed; AV trails exp by
    one tile; epilogue deferred one slice.
"""

import numpy as np
import ml_dtypes

import concourse.bass as bass
import concourse.mybir as mybir
import concourse.tile as tile
from concourse import bacc
from concourse.bass_utils import run_bass_kernel_spmd

B, T, C, H = 8, 2048, 1024, 64
NCB = C // 128          # 8 C-blocks
NT = T // 128           # 16 Tk-blocks of 128
NJ = T // 512           # 4 Tq-slices of 512
SCALE = float(C) ** -0.5  # 1/32

BF16 = mybir.dt.bfloat16
F32 = mybir.dt.float32
npbf16 = ml_dtypes.bfloat16


def build_attention(nc: bass.Bass, tc: tile.TileContext, ctx):
    xT_d = nc.dram_tensor("xT", [128, NCB, T], BF16,
                          kind="ExternalInput").ap()
    wqk_d = nc.dram_tensor("wqk", [128, NCB, 128], BF16,
                           kind="ExternalInput").ap()
    wv_d = nc.dram_tensor("wv", [128, NCB, H], BF16,
                          kind="ExternalInput").ap()
    # idents: cols 0:64 = [I64;I64], 64:192 = causal, 192:257 = I65 pad
    ident_d = nc.dram_tensor("idents", [128, 257], BF16,
                             kind="ExternalInput").ap()
    out_d = nc.dram_tensor("out", [T, H], F32, kind="ExternalOutput").ap()

    consts = ctx.enter_context(tc.tile_pool(name="consts", bufs=1))
    persist = ctx.enter_context(tc.tile_pool(name="persist", bufs=1))
    pts = ctx.enter_context(tc.tile_pool(name="pts", bufs=6))
    outts = ctx.enter_context(tc.tile_pool(name="outts", bufs=2))
    smalls = ctx.enter_context(tc.tile_pool(name="smalls", bufs=2))
    ps_big = ctx.enter_context(tc.tile_pool(name="ps_big", bufs=2,
                                            space="PSUM"))
    ps_v = ctx.enter_context(tc.tile_pool(name="ps_v", bufs=2, space="PSUM"))
    ps_mix = ctx.enter_context(tc.tile_pool(name="ps_mix", bufs=2,
                                            space="PSUM"))

    # ---- input DMAs: minimal count on the two HW DGE queues, emitted in
    # consumption order so the 8-semaphore round-robin reuse chains are
    # always already satisfied.  scalar: wqk + x hi C-half quarters;
    # sync: x lo C-half quarters + wv + idents.
    wqk_sb = consts.tile([128, NCB, 128], BF16, tag="wqk")
    nc.scalar.dma_start(out=wqk_sb, in_=wqk_d)
    xT_sb = persist.tile([128, NCB, T], BF16, tag="xT")
    nc.sync.dma_start(out=xT_sb[:, 0:4, 0:512], in_=xT_d[:, 0:4, 0:512])
    nc.scalar.dma_start(out=xT_sb[:, 4:8, 0:512], in_=xT_d[:, 4:8, 0:512])
    wv_sb = consts.tile([128, NCB, H], BF16, tag="wv")
    nc.sync.dma_start(out=wv_sb, in_=wv_d)
    ident_sb = consts.tile([128, 257], BF16, tag="idents")
    nc.scalar.dma_start(out=ident_sb, in_=ident_d)
    for qa in range(1, 4):
        qs = slice(qa * 512, (qa + 1) * 512)
        nc.sync.dma_start(out=xT_sb[:, 0:4, qs], in_=xT_d[:, 0:4, qs])
        nc.scalar.dma_start(out=xT_sb[:, 4:8, qs], in_=xT_d[:, 4:8, qs])

    i64_sb = ident_sb[:, 0:64]
    causal_sb = ident_sb[:, 64:192]
    i65_sb = ident_sb[0:65, 192:257]

    qkT = persist.tile([128, T], BF16, tag="qkT")      # [q; k]
    qT2hi = persist.tile([128, T], BF16, tag="qT2hi")  # q in rows 64:128
    kT2 = persist.tile([128, T // 2], BF16, tag="kT2")  # Tk pairs in halves
    vT = persist.tile([64, T], BF16, tag="vT")
    vT2 = persist.tile([128, T // 2], BF16, tag="vT2")  # odd Tk blocks, hi
    v1 = persist.tile([128, NT, H + 1], BF16, tag="v1")  # [v | 1]
    nc.vector.memset(v1, 1.0)
    outbuf = persist.tile([128, NT, H], F32, tag="outbuf")

    pending_av = None
    for j in range(NJ):
        jsl = slice(j * 512, (j + 1) * 512)

        # ---- QK projection for slice j --------------------------------
        qk_ps = ps_mix.tile([128, 512], F32, tag="mix", name=f"qk_ps{j}")
        for c in range(NCB):
            nc.tensor.matmul(qk_ps, lhsT=wqk_sb[:, c, :],
                             rhs=xT_sb[:, c, jsl],
                             start=(c == 0), stop=(c == NCB - 1))
        nc.vector.tensor_copy(qkT[:, jsl], qk_ps)
        # odd k-blocks (4j+1, 4j+3) straight from PSUM into kT2 hi half
        for b in (1, 3):
            c0 = (2 * j + b // 2) * 128
            nc.vector.tensor_copy(kT2[64:128, c0:c0 + 128],
                                  qk_ps[64:128, b * 128:(b + 1) * 128])
        # partition-shifted SBUF copies on Pool: q dup, even k-blocks
        nc.gpsimd.tensor_copy(qT2hi[64:128, jsl], qkT[0:64, jsl])
        for b in (0, 2):
            c0 = (2 * j + b // 2) * 128
            nc.gpsimd.tensor_copy(
                kT2[0:64, c0:c0 + 128],
                qkT[64:128, j * 512 + b * 128:j * 512 + (b + 1) * 128])

        # ---- S pair tiles 0,1 hoisted: feed ScalarE exp early ---------
        s_tiles = []
        for m in range(min(2, 2 * j + 2)):
            s_tiles.append(emit_s_tile(nc, ps_big, pts, qkT, qT2hi, kT2,
                                       causal_sb, j, m))

        # ---- V projection for slice j ---------------------------------
        v_ps = ps_v.tile([128, 512], F32, tag="vps", name=f"v_ps{j}")
        for c in range(NCB):
            nc.tensor.matmul(v_ps[0:64, :], lhsT=wv_sb[:, c, :],
                             rhs=xT_sb[:, c, jsl],
                             start=(c == 0), stop=(c == NCB - 1))

        # ---- deferred epilogue of slice j-1 ---------------------------
        if pending_av is not None:
            emit_epilogue(nc, outts, smalls, ps_mix, i65_sb, outbuf,
                          *pending_av)
            pending_av = None
        if j == 3:  # slices 0-2 done: one bulk store for t-blocks 0:12
            nc.sync.dma_start(
                out=out_d[0:12 * 128, :].rearrange("(t p) h -> p t h", p=128),
                in_=outbuf[:, 0:12, :])

        # ---- v-finish: cast, shift, transpose back into v1 ------------
        emit_vfinish(nc, ps_mix, v_ps, vT, vT2, v1, i64_sb, j)

        # ---- attention rest: AV trails exp by one tile ----------------
        av = ps_mix.tile([65, 512], F32, tag="mix", name=f"av{j}")
        nblk = 4 * j + 4
        for m in range(2, 2 * j + 2):
            emit_av(nc, av, v1, *s_tiles[m - 2], nblk)
            s_tiles.append(emit_s_tile(nc, ps_big, pts, qkT, qT2hi, kT2,
                                       causal_sb, j, m))
        for st in s_tiles[-2:]:
            emit_av(nc, av, v1, *st, nblk)
        pending_av = (av, j)

    emit_epilogue(nc, outts, smalls, ps_mix, i65_sb, outbuf, *pending_av,
                  out_d=out_d)


def emit_s_tile(nc, ps_big, pts, qkT, qT2hi, kT2, causal_sb, j, m):
    """Row-packed S^T pair tile (k-blocks 2m, 2m+1) + exp + causal mask."""
    sp2 = ps_big.tile([128, 1024], F32, tag="big", name=f"sp{j}_{m}")
    pt2 = pts.tile([128, 1024], BF16, tag="pt", name=f"pt{j}_{m}")
    n0s = []
    for half_idx, i in ((0, 2 * m), (1, 2 * m + 1)):
        g = i - 4 * j
        n0 = max(0, g) * 128
        p0 = half_idx * 64
        o = half_idx * 512
        rhs = (qkT if half_idx == 0 else qT2hi)
        nc.tensor.matmul(
            sp2[:, o + n0:o + 512],
            lhsT=kT2[p0:p0 + 64, m * 128:(m + 1) * 128],
            rhs=rhs[p0:p0 + 64, j * 512 + n0:(j + 1) * 512],
            start=True, stop=True)
        n0s.append(n0)
    if n0s[0] == 0 and n0s[1] == 0:  # one wide exp over both banks
        nc.scalar.activation(pt2, sp2, mybir.ActivationFunctionType.Exp,
                             scale=SCALE)
    else:
        for half_idx in range(2):
            o, n0 = half_idx * 512, n0s[half_idx]
            nc.scalar.activation(
                pt2[:, o + n0:o + 512], sp2[:, o + n0:o + 512],
                mybir.ActivationFunctionType.Exp, scale=SCALE)
    for half_idx, i in ((0, 2 * m), (1, 2 * m + 1)):
        g = i - 4 * j
        if g >= 0:  # mask upper triangle of the diagonal block
            o = half_idx * 512 + n0s[half_idx]
            nc.gpsimd.tensor_mul(
                pt2[:, o:o + 128], pt2[:, o:o + 128], causal_sb)
    return (pt2, n0s, 2 * m)


def emit_vfinish(nc, ps_mix, v_ps, vT, vT2, v1, i64_sb, j):
    """v^T cast, odd-block shift, and transpose back into v1."""
    jsl = slice(j * 512, (j + 1) * 512)
    nc.vector.tensor_copy(vT[:, jsl], v_ps[0:64, :])
    # odd Tk blocks 4j+1, 4j+3 -> vT2 hi half (partition-shifted on Pool)
    for bb in range(2):
        tb = 4 * j + 2 * bb + 1
        c0 = (2 * j + bb) * 128
        nc.gpsimd.tensor_copy(vT2[64:128, c0:c0 + 128],
                              vT[:, tb * 128:(tb + 1) * 128])
    # v natural via row-packed identity matmuls (pair of Tk blocks)
    for mt in (2 * j, 2 * j + 1):
        tA, tB = 2 * mt, 2 * mt + 1
        vpA = ps_mix.tile([128, H], F32, tag="mix", name=f"vpA{mt}")
        vpB = ps_mix.tile([128, H], F32, tag="mix", name=f"vpB{mt}")
        nc.tensor.matmul(vpA, lhsT=vT[:, tA * 128:(tA + 1) * 128],
                         rhs=i64_sb[0:64, :], start=True, stop=True)
        nc.tensor.matmul(vpB, lhsT=vT2[64:128, mt * 128:(mt + 1) * 128],
                         rhs=i64_sb[64:128, :], start=True, stop=True)
        nc.vector.tensor_copy(v1[:, tA, 0:H], vpA)
        nc.vector.tensor_copy(v1[:, tB, 0:H], vpB)


def emit_av(nc, av, v1, pt2, n0s, i0, nblk):
    for d in range(2):
        i = i0 + d
        o, n0 = d * 512, n0s[d]
        nc.tensor.matmul(av[:, n0:512], lhsT=v1[:, i, :],
                         rhs=pt2[:, o + n0:o + 512],
                         start=(i == 0), stop=(i == nblk - 1))


def emit_epilogue(nc, outts, smalls, ps_mix, i65_sb, outbuf, av, j,
                  out_d=None):
    osb = outts.tile([65, 512], BF16, tag="osb", name=f"osb{j}")
    nc.vector.tensor_copy(osb, av)  # f32 PSUM -> bf16 SBUF
    for t in range(4):
        op = ps_mix.tile([128, H + 1], F32, tag="mix", name=f"op{j}_{t}")
        nc.tensor.matmul(op, lhsT=osb[:, t * 128:(t + 1) * 128], rhs=i65_sb,
                         start=True, stop=True)
        rc = smalls.tile([128, 1], F32, tag="rc", name=f"rc{j}_{t}")
        nc.vector.reciprocal(rc, op[:, H:H + 1])
        tb = j * 4 + t
        nc.vector.tensor_scalar_mul(outbuf[:, tb, :], op[:, 0:H], rc)
        if out_d is not None:  # last slice: store per block for a short tail
            nc.sync.dma_start(out=out_d[tb * 128:(tb + 1) * 128, :],
                              in_=outbuf[:, tb, :])


_CACHED = {}


def _get_nc():
    if "nc" not in _CACHED:
        from contextlib import ExitStack
        nc = bacc.Bacc("TRN2", target_bir_lowering=False, debug=False,
                       num_devices=B)
        with tile.TileContext(nc) as tc:
            with ExitStack() as ctx:
                build_attention(nc, tc, ctx)
        nc.compile()
        _CACHED["nc"] = nc
    return _CACHED["nc"]


def _quant_inputs(inputs, Wq, Wk, Wv):
    """Host-side prep: xT in [128, 8, T] bf16 layout, packed [Wq|Wk]."""
    inputs = np.asarray(inputs, dtype=np.float32)

    def wlayout(w, m):  # [C, m] -> [128, 8, m]
        return np.ascontiguousarray(
            np.asarray(w).astype(npbf16).reshape(8, 128, m).transpose(
                1, 0, 2))

    wqk = wlayout(np.concatenate([np.asarray(Wq), np.asarray(Wk)], axis=1),
                  128)
    wv = wlayout(Wv, H)

    idents = np.zeros((128, 257), dtype=npbf16)
    idents[0:64, 0:64] = np.eye(64, dtype=npbf16)
    idents[64:128, 0:64] = np.eye(64, dtype=npbf16)
    idents[:, 64:192] = np.triu(np.ones((128, 128), dtype=npbf16))
    idents[0:65, 192:257] = np.eye(65, dtype=npbf16)

    in_maps = []
    for b in range(B):
        xT = np.ascontiguousarray(
            inputs[b].T.astype(npbf16).reshape(8, 128, T).transpose(1, 0, 2))
        in_maps.append({"xT": xT, "wqk": wqk, "wv": wv, "idents": idents})
    return in_maps


def kernel(inputs, Wq, Wk, Wv):
    in_maps = _quant_inputs(inputs, Wq, Wk, Wv)
    nc = _get_nc()
    res = run_bass_kernel_spmd(nc, in_maps, core_ids=list(range(B)))
    out = np.stack([res.results[b]["out"] for b in range(B)], axis=0)
    return out.astype(np.float32)


# revision 18
# speedup vs baseline: 1.2597x; 1.0327x over previous
"""Single-head causal attention on 8 TRN2 NeuronCores.

Problem shapes (hardcoded): B=8, T=2048, C=1024, H=64, fp32 I/O.
    q = x @ Wq; k = x @ Wk; v = x @ Wv          (per batch element)
    wei = softmax(causal_mask(q @ k.T * C**-0.5))
    out = wei @ v
Sharding: pure data parallel - one batch element per core, no collectives.

Per-core algorithm (bf16 matmuls, fp32 PSUM accumulation):
  - host pre-transposes x -> xT [C, T] (C on partitions) and packs
    [Wq|Wk]; projections run per 512-wide T-slice: qkT = [Wq|Wk].T @ xT,
    vT = Wv.T @ xT.  (fp8 DoubleRow was tried and measured SLOWER: each
    matmul pays its own LDWEIGHTS, and a 256-col DR weight load outruns
    the halved 107ns stream time, 259ns/MM vs bf16's 216ns/MM.)
  - S^T row-packed: kT2 holds Tk-block pairs in the two partition halves,
    qT2hi duplicates q into the hi half; h0 reads q straight from the
    projection cast (qkT rows 0:64).  All half-shuffles are
    partition-shifted ENGINE copies, not DMAs: Pool shifts SBUF->SBUF,
    DVE drains PSUM.  P = exp(S/32), no max-subtraction (logits std
    ~0.25); causal diag blocks masked by a 0/1 multiply on Pool.
  - v1 = [v | 1] so [num|den] share one accumulator; v natural recovered
    from vT by row-packed identity matmuls.
  - HW-DGE DMA queues (sync/scalar) carry ONLY input streams + output
    stores (~16 instructions): the two queues share 8 completion
    semaphores, and extra DMAs build false cross-queue serialization
    chains.  xT streams in T-quarter x C-half chunks so slice j's
    projections unblock at ~2.7(j+1)us.  Outputs collect in one SBUF
    buffer; slices 0-2 store as one DMA, slice 3 per-block (short tail).
  - per-slice pipeline: QK proj -> casts/shifts -> S pair tiles 0,1
    hoisted BEFORE the V projection so ScalarE exp (the ~17us secondary
    bottleneck; PE ~25us) starts early and stays fed; AV trails exp by
    one tile; epilogue deferred one slice.
"""

import numpy as np
import ml_dtypes

import concourse.bass as bass
import concourse.mybir as mybir
import concourse.tile as tile
from concourse import bacc
from concourse.bass_utils import run_bass_kernel_spmd

B, T, C, H = 8, 2048, 1024, 64
NCB = C // 128          # 8 C-blocks
NT = T // 128           # 16 Tk-blocks of 128
NJ = T // 512           # 4 Tq-slices of 512
SCALE = float(C) ** -0.5  # 1/32

BF16 = mybir.dt.bfloat16
F32 = mybir.dt.float32
npbf16 = ml_dtypes.bfloat16


def build_attention(nc: bass.Bass, tc: tile.TileContext, ctx):
    xT_d = nc.dram_tensor("xT", [128, NCB, T], BF16,
                          kind="ExternalInput").ap()
    wqk_d = nc.dram_tensor("wqk", [128, NCB, 128], BF16,
                           kind="ExternalInput").ap()
    wv_d = nc.dram_tensor("wv", [128, NCB, H], BF16,
                          kind="ExternalInput").ap()
    # idents: cols 0:64 = [I64;I64], 64:192 = causal, 192:257 = I65 pad
    ident_d = nc.dram_tensor("idents", [128, 257], BF16,
                             kind="ExternalInput").ap()
    out_d = nc.dram_tensor("out", [T, H], F32, kind="ExternalOutput").ap()

    consts = ctx.enter_context(tc.tile_pool(name="consts", bufs=1))
    persist = ctx.enter_context(tc.tile_pool(name="persist", bufs=1))
    pts = ctx.enter_context(tc.tile_pool(name="pts", bufs=6))
    outts = ctx.enter_context(tc.tile_pool(name="outts", bufs=2))
    smalls = ctx.enter_context(tc.tile_pool(name="smalls", bufs=2))
    ps_big = ctx.enter_context(tc.tile_pool(name="ps_big", bufs=2,
                                            space="PSUM"))
    ps_v = ctx.enter_context(tc.tile_pool(name="ps_v", bufs=2, space="PSUM"))
    ps_mix = ctx.enter_context(tc.tile_pool(name="ps_mix", bufs=2,
                                            space="PSUM"))

    # ---- input DMAs: minimal count on the two HW DGE queues, emitted in
    # consumption order so the 8-semaphore round-robin reuse chains are
    # always already satisfied.  scalar: wqk + x hi C-half quarters;
    # sync: x lo C-half quarters + wv + idents.
    wqk_sb = consts.tile([128, NCB, 128], BF16, tag="wqk")
    nc.scalar.dma_start(out=wqk_sb, in_=wqk_d)
    xT_sb = persist.tile([128, NCB, T], BF16, tag="xT")
    nc.sync.dma_start(out=xT_sb[:, 0:4, 0:512], in_=xT_d[:, 0:4, 0:512])
    nc.scalar.dma_start(out=xT_sb[:, 4:8, 0:512], in_=xT_d[:, 4:8, 0:512])
    wv_sb = consts.tile([128, NCB, H], BF16, tag="wv")
    nc.sync.dma_start(out=wv_sb, in_=wv_d)
    ident_sb = consts.tile([128, 257], BF16, tag="idents")
    nc.scalar.dma_start(out=ident_sb, in_=ident_d)
    for qa in range(1, 4):
        qs = slice(qa * 512, (qa + 1) * 512)
        nc.sync.dma_start(out=xT_sb[:, 0:4, qs], in_=xT_d[:, 0:4, qs])
        nc.scalar.dma_start(out=xT_sb[:, 4:8, qs], in_=xT_d[:, 4:8, qs])

    i64_sb = ident_sb[:, 0:64]
    causal_sb = ident_sb[:, 64:192]
    i65_sb = ident_sb[0:65, 192:257]

    qkT = persist.tile([128, T], BF16, tag="qkT")      # [q; k]
    qT2hi = persist.tile([128, T], BF16, tag="qT2hi")  # q in rows 64:128
    kT2 = persist.tile([128, T // 2], BF16, tag="kT2")  # Tk pairs in halves
    vT = persist.tile([64, T], BF16, tag="vT")
    vT2 = persist.tile([128, T // 2], BF16, tag="vT2")  # odd Tk blocks, hi
    v1 = persist.tile([128, NT, H + 1], BF16, tag="v1")  # [v | 1]
    nc.vector.memset(v1, 1.0)
    outbuf = persist.tile([128, NT, H], F32, tag="outbuf")

    # The AV/epilogue pipeline lags the projection/S pipeline by a full
    # slice: slice j's section runs QK_j/V_j/S_j(0,1) so the PE always has
    # projection work while ScalarE chews exp, then finishes slice j-1's
    # AVs.  Epilogues lag two slices.
    s_pend = [None] * NJ
    avs = [None] * NJ

    def att_rest(i):
        avs[i] = ps_mix.tile([65, 512], F32, tag="mix", name=f"av{i}")
        nblk = 4 * i + 4
        for m in range(2, 2 * i + 2):
            emit_av(nc, avs[i], v1, *s_pend[i][m - 2], nblk)
            s_pend[i].append(emit_s_tile(nc, ps_big, pts, qkT, qT2hi, kT2,
                                         causal_sb, i, m))
        for st in s_pend[i][-2:]:
            emit_av(nc, avs[i], v1, *st, nblk)

    for j in range(NJ):
        jsl = slice(j * 512, (j + 1) * 512)

        # ---- QK projection for slice j --------------------------------
        qk_ps = ps_mix.tile([128, 512], F32, tag="mix", name=f"qk_ps{j}")
        for c in range(NCB):
            nc.tensor.matmul(qk_ps, lhsT=wqk_sb[:, c, :],
                             rhs=xT_sb[:, c, jsl],
                             start=(c == 0), stop=(c == NCB - 1))
        nc.vector.tensor_copy(qkT[:, jsl], qk_ps)
        # odd k-blocks (4j+1, 4j+3) straight from PSUM into kT2 hi half
        for b in (1, 3):
            c0 = (2 * j + b // 2) * 128
            nc.vector.tensor_copy(kT2[64:128, c0:c0 + 128],
                                  qk_ps[64:128, b * 128:(b + 1) * 128])
        # partition-shifted SBUF copies on Pool: q dup, even k-blocks
        nc.gpsimd.tensor_copy(qT2hi[64:128, jsl], qkT[0:64, jsl])
        for b in (0, 2):
            c0 = (2 * j + b // 2) * 128
            nc.gpsimd.tensor_copy(
                kT2[0:64, c0:c0 + 128],
                qkT[64:128, j * 512 + b * 128:j * 512 + (b + 1) * 128])

        # ---- V projection for slice j ---------------------------------
        v_ps = ps_v.tile([128, 512], F32, tag="vps", name=f"v_ps{j}")
        for c in range(NCB):
            nc.tensor.matmul(v_ps[0:64, :], lhsT=wv_sb[:, c, :],
                             rhs=xT_sb[:, c, jsl],
                             start=(c == 0), stop=(c == NCB - 1))

        # ---- S pair tiles 0,1: feed ScalarE exp early -----------------
        s_pend[j] = [emit_s_tile(nc, ps_big, pts, qkT, qT2hi, kT2,
                                 causal_sb, j, m) for m in range(2)]

        if j >= 2:  # epilogue of slice j-2
            emit_epilogue(nc, outts, smalls, ps_mix, i65_sb, outbuf,
                          avs[j - 2], j - 2)

        # ---- v-finish: cast, shift, transpose back into v1 ------------
        emit_vfinish(nc, ps_mix, v_ps, vT, vT2, v1, i64_sb, j)

        if j >= 1:  # finish slice j-1's attention
            att_rest(j - 1)

    emit_epilogue(nc, outts, smalls, ps_mix, i65_sb, outbuf, avs[2], 2)
    nc.sync.dma_start(  # slices 0-2 done: one bulk store for blocks 0:12
        out=out_d[0:12 * 128, :].rearrange("(t p) h -> p t h", p=128),
        in_=outbuf[:, 0:12, :])
    att_rest(3)
    emit_epilogue(nc, outts, smalls, ps_mix, i65_sb, outbuf, avs[3], 3,
                  out_d=out_d)


def emit_s_tile(nc, ps_big, pts, qkT, qT2hi, kT2, causal_sb, j, m):
    """Row-packed S^T pair tile (k-blocks 2m, 2m+1) + exp + causal mask."""
    sp2 = ps_big.tile([128, 1024], F32, tag="big", name=f"sp{j}_{m}")
    pt2 = pts.tile([128, 1024], BF16, tag="pt", name=f"pt{j}_{m}")
    n0s = []
    for half_idx, i in ((0, 2 * m), (1, 2 * m + 1)):
        g = i - 4 * j
        n0 = max(0, g) * 128
        p0 = half_idx * 64
        o = half_idx * 512
        rhs = (qkT if half_idx == 0 else qT2hi)
        nc.tensor.matmul(
            sp2[:, o + n0:o + 512],
            lhsT=kT2[p0:p0 + 64, m * 128:(m + 1) * 128],
            rhs=rhs[p0:p0 + 64, j * 512 + n0:(j + 1) * 512],
            start=True, stop=True)
        n0s.append(n0)
    if n0s[0] == 0 and n0s[1] == 0:  # one wide exp over both banks
        nc.scalar.activation(pt2, sp2, mybir.ActivationFunctionType.Exp,
                             scale=SCALE)
    else:
        for half_idx in range(2):
            o, n0 = half_idx * 512, n0s[half_idx]
            nc.scalar.activation(
                pt2[:, o + n0:o + 512], sp2[:, o + n0:o + 512],
                mybir.ActivationFunctionType.Exp, scale=SCALE)
    for half_idx, i in ((0, 2 * m), (1, 2 * m + 1)):
        g = i - 4 * j
        if g >= 0:  # mask upper triangle of the diagonal block
            o = half_idx * 512 + n0s[half_idx]
            nc.gpsimd.tensor_mul(
                pt2[:, o:o + 128], pt2[:, o:o + 128], causal_sb)
    return (pt2, n0s, 2 * m)


def emit_vfinish(nc, ps_mix, v_ps, vT, vT2, v1, i64_sb, j):
    """v^T cast, odd-block shift, and transpose back into v1."""
    jsl = slice(j * 512, (j + 1) * 512)
    nc.vector.tensor_copy(vT[:, jsl], v_ps[0:64, :])
    # odd Tk blocks 4j+1, 4j+3 -> vT2 hi half (partition-shifted on Pool)
    for bb in range(2):
        tb = 4 * j + 2 * bb + 1
        c0 = (2 * j + bb) * 128
        nc.gpsimd.tensor_copy(vT2[64:128, c0:c0 + 128],
                              vT[:, tb * 128:(tb + 1) * 128])
    # v natural via row-packed identity matmuls (pair of Tk blocks)
    for mt in (2 * j, 2 * j + 1):
        tA, tB = 2 * mt, 2 * mt + 1
        vpA = ps_mix.tile([128, H], F32, tag="mix", name=f"vpA{mt}")
        vpB = ps_mix.tile([128, H], F32, tag="mix", name=f"vpB{mt}")
        nc.tensor.matmul(vpA, lhsT=vT[:, tA * 128:(tA + 1) * 128],
                         rhs=i64_sb[0:64, :], start=True, stop=True)
        nc.tensor.matmul(vpB, lhsT=vT2[64:128, mt * 128:(mt + 1) * 128],
                         rhs=i64_sb[64:128, :], start=True, stop=True)
        nc.vector.tensor_copy(v1[:, tA, 0:H], vpA)
        nc.vector.tensor_copy(v1[:, tB, 0:H], vpB)


def emit_av(nc, av, v1, pt2, n0s, i0, nblk):
    for d in range(2):
        i = i0 + d
        o, n0 = d * 512, n0s[d]
        nc.tensor.matmul(av[:, n0:512], lhsT=v1[:, i, :],
                         rhs=pt2[:, o + n0:o + 512],
                         start=(i == 0), stop=(i == nblk - 1))


def emit_epilogue(nc, outts, smalls, ps_mix, i65_sb, outbuf, av, j,
                  out_d=None):
    osb = outts.tile([65, 512], BF16, tag="osb", name=f"osb{j}")
    nc.vector.tensor_copy(osb, av)  # f32 PSUM -> bf16 SBUF
    for t in range(4):
        op = ps_mix.tile([128, H + 1], F32, tag="mix", name=f"op{j}_{t}")
        nc.tensor.matmul(op, lhsT=osb[:, t * 128:(t + 1) * 128], rhs=i65_sb,
                         start=True, stop=True)
        rc = smalls.tile([128, 1], F32, tag="rc", name=f"rc{j}_{t}")
        nc.vector.reciprocal(rc, op[:, H:H + 1])
        tb = j * 4 + t
        nc.vector.tensor_scalar_mul(outbuf[:, tb, :], op[:, 0:H], rc)
        if out_d is not None:  # last slice: store per block for a short tail
            nc.sync.dma_start(out=out_d[tb * 128:(tb + 1) * 128, :],
                              in_=outbuf[:, tb, :])


_CACHED = {}


def _get_nc():
    if "nc" not in _CACHED:
        from contextlib import ExitStack
        nc = bacc.Bacc("TRN2", target_bir_lowering=False, debug=False,
                       num_devices=B)
        with tile.TileContext(nc) as tc:
            with ExitStack() as ctx:
                build_attention(nc, tc, ctx)
        nc.compile()
        _CACHED["nc"] = nc
    return _CACHED["nc"]


def _quant_inputs(inputs, Wq, Wk, Wv):
    """Host-side prep: xT in [128, 8, T] bf16 layout, packed [Wq|Wk]."""
    inputs = np.asarray(inputs, dtype=np.float32)

    def wlayout(w, m):  # [C, m] -> [128, 8, m]
        return np.ascontiguousarray(
            np.asarray(w).astype(npbf16).reshape(8, 128, m).transpose(
                1, 0, 2))

    wqk = wlayout(np.concatenate([np.asarray(Wq), np.asarray(Wk)], axis=1),
                  128)
    wv = wlayout(Wv, H)

    idents = np.zeros((128, 257), dtype=npbf16)
    idents[0:64, 0:64] = np.eye(64, dtype=npbf16)
    idents[64:128, 0:64] = np.eye(64, dtype=npbf16)
    idents[:, 64:192] = np.triu(np.ones((128, 128), dtype=npbf16))
    idents[0:65, 192:257] = np.eye(65, dtype=npbf16)

    in_maps = []
    for b in range(B):
        xT = np.ascontiguousarray(
            inputs[b].T.astype(npbf16).reshape(8, 128, T).transpose(1, 0, 2))
        in_maps.append({"xT": xT, "wqk": wqk, "wv": wv, "idents": idents})
    return in_maps


def kernel(inputs, Wq, Wk, Wv):
    in_maps = _quant_inputs(inputs, Wq, Wk, Wv)
    nc = _get_nc()
    res = run_bass_kernel_spmd(nc, in_maps, core_ids=list(range(B)))
    out = np.stack([res.results[b]["out"] for b in range(B)], axis=0)
    return out.astype(np.float32)


# revision 19
# speedup vs baseline: 1.3476x; 1.0698x over previous
"""Single-head causal attention on 8 TRN2 NeuronCores.

Problem shapes (hardcoded): B=8, T=2048, C=1024, H=64, fp32 I/O.
    q = x @ Wq; k = x @ Wk; v = x @ Wv          (per batch element)
    wei = softmax(causal_mask(q @ k.T * C**-0.5))
    out = wei @ v
Sharding: pure data parallel - one batch element per core, no collectives.

Per-core algorithm (bf16 matmuls, fp32 PSUM accumulation):
  - host pre-transposes x -> xT [C, T] and packs [Wq|Wk]; per 512-wide
    T-slice: qkT = [Wq|Wk].T @ xT, vT = Wv.T @ xT.  (fp8 DoubleRow was
    tried and measured SLOWER: every matmul pays its own LDWEIGHTS and a
    256-col DR weight load outruns the halved stream time.)
  - S^T row-packed: kT2 holds Tk-block pairs in the partition halves,
    qT2hi duplicates q into the hi half; h0 reads q straight from qkT.
    All half-shuffles are partition-shifted ENGINE copies (Pool for
    SBUF->SBUF, DVE to drain PSUM) - DMAs here would serialize on the
    8 shared HW-DGE semaphores.
  - exp always one WIDE ACT per [128,1024] pair tile; columns outside
    the causal n0 window hold garbage that AV never reads.  P = exp(S/32)
    with no max-subtraction; diagonal blocks masked 0/1 on Pool.
  - v1 = [v | 1] -> [num|den] share one accumulator; v natural recovered
    by row-packed identity matmuls.
  - THE SCHEDULE IS A FLAT GLOBAL INTERLEAVE: ScalarE exp (~20 x 1us) is
    the secondary bottleneck, so S pair tiles are emitted one per ~1us of
    other PE work (projections / AV / transposes / epilogues as filler).
    PSUM pools rotate deadlock-free: ps_big = S pairs (2 bufs), ps_v =
    v_ps + av alternating, ps_mix = qk / v-transpose / epilogue tiles.
  - 8 dummy warmup matmuls release the HAM clock gate (PE starts at
    1.2 GHz, reaches 2.4 only after ~3.4us of sustained activity) while
    the input DMAs stream.
  - HW-DGE queues carry only inputs + stores (~16 DMA instructions);
    xT streams in T-quarter x C-half chunks, outputs collect in SBUF
    (one bulk store for slices 0-2, per-block stores for slice 3).
"""

import numpy as np
import ml_dtypes

import concourse.bass as bass
import concourse.mybir as mybir
import concourse.tile as tile
from concourse import bacc
from concourse.bass_utils import run_bass_kernel_spmd

B, T, C, H = 8, 2048, 1024, 64
NCB = C // 128          # 8 C-blocks
NT = T // 128           # 16 Tk-blocks of 128
NJ = T // 512           # 4 Tq-slices of 512
SCALE = float(C) ** -0.5  # 1/32

BF16 = mybir.dt.bfloat16
F32 = mybir.dt.float32
npbf16 = ml_dtypes.bfloat16


class Ctx:
    pass


def build_attention(nc: bass.Bass, tc: tile.TileContext, ctx):
    g = Ctx()
    g.nc = nc
    xT_d = nc.dram_tensor("xT", [128, NCB, T], BF16,
                          kind="ExternalInput").ap()
    wqk_d = nc.dram_tensor("wqk", [128, NCB, 128], BF16,
                           kind="ExternalInput").ap()
    wv_d = nc.dram_tensor("wv", [128, NCB, H], BF16,
                          kind="ExternalInput").ap()
    ident_d = nc.dram_tensor("idents", [128, 257], BF16,
                             kind="ExternalInput").ap()
    g.out_d = nc.dram_tensor("out", [T, H], F32, kind="ExternalOutput").ap()

    consts = ctx.enter_context(tc.tile_pool(name="consts", bufs=1))
    persist = ctx.enter_context(tc.tile_pool(name="persist", bufs=1))
    g.pts = ctx.enter_context(tc.tile_pool(name="pts", bufs=6))
    g.outts = ctx.enter_context(tc.tile_pool(name="outts", bufs=2))
    g.smalls = ctx.enter_context(tc.tile_pool(name="smalls", bufs=2))
    g.ps_big = ctx.enter_context(tc.tile_pool(name="ps_big", bufs=2,
                                              space="PSUM"))
    g.ps_v = ctx.enter_context(tc.tile_pool(name="ps_v", bufs=2,
                                            space="PSUM"))
    g.ps_mix = ctx.enter_context(tc.tile_pool(name="ps_mix", bufs=2,
                                              space="PSUM"))

    # ---- input DMAs: minimal count on the two HW DGE queues, in
    # consumption order (they share 8 completion semaphores round-robin).
    g.wqk_sb = consts.tile([128, NCB, 128], BF16, tag="wqk")
    nc.scalar.dma_start(out=g.wqk_sb[:, 0:4, :], in_=wqk_d[:, 0:4, :])
    nc.scalar.dma_start(out=g.wqk_sb[:, 4:8, :], in_=wqk_d[:, 4:8, :])
    g.xT_sb = persist.tile([128, NCB, T], BF16, tag="xT")
    nc.sync.dma_start(out=g.xT_sb[:, 0:4, 0:512], in_=xT_d[:, 0:4, 0:512])
    nc.scalar.dma_start(out=g.xT_sb[:, 4:8, 0:512], in_=xT_d[:, 4:8, 0:512])
    g.wv_sb = consts.tile([128, NCB, H], BF16, tag="wv")
    ident_sb = consts.tile([128, 257], BF16, tag="idents")
    nc.scalar.dma_start(out=ident_sb, in_=ident_d)
    nc.sync.dma_start(out=g.xT_sb[:, 0:4, 512:1024],
                      in_=xT_d[:, 0:4, 512:1024])
    nc.sync.dma_start(out=g.wv_sb, in_=wv_d)
    nc.scalar.dma_start(out=g.xT_sb[:, 4:8, 512:1024],
                        in_=xT_d[:, 4:8, 512:1024])
    for qa in range(2, 4):
        qs = slice(qa * 512, (qa + 1) * 512)
        nc.sync.dma_start(out=g.xT_sb[:, 0:4, qs], in_=xT_d[:, 0:4, qs])
        nc.scalar.dma_start(out=g.xT_sb[:, 4:8, qs], in_=xT_d[:, 4:8, qs])

    g.i64_sb = ident_sb[:, 0:64]
    g.causal_sb = ident_sb[:, 64:192]
    g.i65_sb = ident_sb[0:65, 192:257]

    g.qkT = persist.tile([128, T], BF16, tag="qkT")      # [q; k]
    g.qT2hi = persist.tile([128, T], BF16, tag="qT2hi")  # q in rows 64:128
    g.kT2 = persist.tile([128, T // 2], BF16, tag="kT2")
    g.vT = persist.tile([64, T], BF16, tag="vT")
    g.vT2 = persist.tile([128, T // 2], BF16, tag="vT2")
    g.v1 = persist.tile([128, NT, H + 1], BF16, tag="v1")  # [v | 1]
    nc.vector.memset(g.v1, 1.0)
    g.outbuf = persist.tile([128, NT, H], F32, tag="outbuf")

    g.s_pend = [[] for _ in range(NJ)]
    g.avs = [None] * NJ

    # ---- flat global schedule ------------------------------------------
    QK, V, S, VTR, AV, EP = (emit_qk, emit_v, emit_s, emit_vtr, emit_avu,
                             emit_ep)
    QK(g, 0, warmup=8)
    V(g, 0)
    S(g, 0, 0)
    VTR(g, 0)
    S(g, 0, 1)
    QK(g, 1)
    AV(g, 0, 0)
    S(g, 1, 0)
    AV(g, 0, 1)
    S(g, 1, 1)
    EP(g, 0)
    V(g, 1)
    S(g, 1, 2)
    VTR(g, 1)
    S(g, 1, 3)
    QK(g, 2)
    AV(g, 1, 0)
    AV(g, 1, 1)
    S(g, 2, 0)
    AV(g, 1, 2)
    AV(g, 1, 3)
    S(g, 2, 1)
    EP(g, 1)
    V(g, 2)
    S(g, 2, 2)
    VTR(g, 2)
    S(g, 2, 3)
    QK(g, 3)
    S(g, 2, 4)
    AV(g, 2, 0)
    AV(g, 2, 1)
    S(g, 2, 5)
    V(g, 3)
    S(g, 3, 0)
    AV(g, 2, 2)
    AV(g, 2, 3)
    S(g, 3, 1)
    VTR(g, 3)
    S(g, 3, 2)
    AV(g, 2, 4)
    AV(g, 2, 5)
    S(g, 3, 3)
    EP(g, 2)
    nc.sync.dma_start(  # slices 0-2 done: bulk store for t-blocks 0:12
        out=g.out_d[0:12 * 128, :].rearrange("(t p) h -> p t h", p=128),
        in_=g.outbuf[:, 0:12, :])
    S(g, 3, 4)
    AV(g, 3, 0)
    AV(g, 3, 1)
    S(g, 3, 5)
    AV(g, 3, 2)
    AV(g, 3, 3)
    S(g, 3, 6)
    S(g, 3, 7)
    AV(g, 3, 4)
    AV(g, 3, 5)
    AV(g, 3, 6)
    AV(g, 3, 7)
    EP(g, 3, store=True)


def emit_qk(g, j, warmup=0):
    nc = g.nc
    jsl = slice(j * 512, (j + 1) * 512)
    qk_ps = g.ps_mix.tile([128, 512], F32, tag="mix", name=f"qk_ps{j}")
    for w in range(warmup):  # HAM warmup; first real matmul resets PSUM
        nc.tensor.matmul(qk_ps[0:65, 0:455], lhsT=g.v1[:, 0, :],
                         rhs=g.v1[:, w:w + 7, :], start=True, stop=True,
                         skip_group_check=True)
    for c in range(NCB):
        nc.tensor.matmul(qk_ps, lhsT=g.wqk_sb[:, c, :],
                         rhs=g.xT_sb[:, c, jsl],
                         start=(c == 0), stop=(c == NCB - 1))
    nc.vector.tensor_copy(g.qkT[:, jsl], qk_ps)
    # odd k-blocks (4j+1, 4j+3) straight from PSUM into kT2 hi half
    for b in (1, 3):
        c0 = (2 * j + b // 2) * 128
        nc.vector.tensor_copy(g.kT2[64:128, c0:c0 + 128],
                              qk_ps[64:128, b * 128:(b + 1) * 128])
    # partition-shifted SBUF copies on Pool: q dup, even k-blocks
    nc.gpsimd.tensor_copy(g.qT2hi[64:128, jsl], g.qkT[0:64, jsl])
    for b in (0, 2):
        c0 = (2 * j + b // 2) * 128
        nc.gpsimd.tensor_copy(
            g.kT2[0:64, c0:c0 + 128],
            g.qkT[64:128, j * 512 + b * 128:j * 512 + (b + 1) * 128])


def emit_v(g, j):
    nc = g.nc
    jsl = slice(j * 512, (j + 1) * 512)
    g.v_ps = g.ps_v.tile([128, 512], F32, tag="vps", name=f"v_ps{j}")
    for c in range(NCB):
        nc.tensor.matmul(g.v_ps[0:64, :], lhsT=g.wv_sb[:, c, :],
                         rhs=g.xT_sb[:, c, jsl],
                         start=(c == 0), stop=(c == NCB - 1))


def emit_s(g, j, m):
    """Row-packed S^T pair tile (k-blocks 2m, 2m+1): one wide exp."""
    nc = g.nc
    sp2 = g.ps_big.tile([128, 1024], F32, tag="big", name=f"sp{j}_{m}")
    pt2 = g.pts.tile([128, 1024], BF16, tag="pt", name=f"pt{j}_{m}")
    n0s = []
    for half_idx, i in ((0, 2 * m), (1, 2 * m + 1)):
        g_ = i - 4 * j
        n0 = max(0, g_) * 128
        p0 = half_idx * 64
        o = half_idx * 512
        rhs = (g.qkT if half_idx == 0 else g.qT2hi)
        nc.tensor.matmul(
            sp2[:, o + n0:o + 512],
            lhsT=g.kT2[p0:p0 + 64, m * 128:(m + 1) * 128],
            rhs=rhs[p0:p0 + 64, j * 512 + n0:(j + 1) * 512],
            start=True, stop=True)
        n0s.append(n0)
    # wide exp over both banks; columns below n0 are garbage nobody reads
    nc.scalar.activation(pt2, sp2, mybir.ActivationFunctionType.Exp,
                         scale=SCALE)
    for half_idx, i in ((0, 2 * m), (1, 2 * m + 1)):
        if i - 4 * j >= 0:  # mask upper triangle of the diagonal block
            o = half_idx * 512 + n0s[half_idx]
            nc.gpsimd.tensor_mul(
                pt2[:, o:o + 128], pt2[:, o:o + 128], g.causal_sb)
    g.s_pend[j].append((pt2, n0s, 2 * m))


def emit_vtr(g, j):
    """v^T cast, odd-block shift, transpose back into v1 = [v|1]."""
    nc = g.nc
    jsl = slice(j * 512, (j + 1) * 512)
    nc.vector.tensor_copy(g.vT[:, jsl], g.v_ps[0:64, :])
    for bb in range(2):  # odd Tk blocks -> vT2 hi half (Pool shift)
        tb = 4 * j + 2 * bb + 1
        c0 = (2 * j + bb) * 128
        nc.gpsimd.tensor_copy(g.vT2[64:128, c0:c0 + 128],
                              g.vT[:, tb * 128:(tb + 1) * 128])
    for mt in (2 * j, 2 * j + 1):
        tA, tB = 2 * mt, 2 * mt + 1
        vpA = g.ps_mix.tile([128, H], F32, tag="mix", name=f"vpA{mt}")
        vpB = g.ps_mix.tile([128, H], F32, tag="mix", name=f"vpB{mt}")
        nc.tensor.matmul(vpA, lhsT=g.vT[:, tA * 128:(tA + 1) * 128],
                         rhs=g.i64_sb[0:64, :], start=True, stop=True)
        nc.tensor.matmul(vpB, lhsT=g.vT2[64:128, mt * 128:(mt + 1) * 128],
                         rhs=g.i64_sb[64:128, :], start=True, stop=True)
        nc.vector.tensor_copy(g.v1[:, tA, 0:H], vpA)
        nc.vector.tensor_copy(g.v1[:, tB, 0:H], vpB)


def emit_avu(g, j, m):
    """AV accumulation for pair tile m of slice j (2 matmuls)."""
    nc = g.nc
    if m == 0:
        g.avs[j] = g.ps_v.tile([65, 512], F32, tag="vps", name=f"av{j}")
    av = g.avs[j]
    pt2, n0s, i0 = g.s_pend[j][m]
    nblk = 4 * j + 4
    for d in range(2):
        i = i0 + d
        o, n0 = d * 512, n0s[d]
        nc.tensor.matmul(av[:, n0:512], lhsT=g.v1[:, i, :],
                         rhs=pt2[:, o + n0:o + 512],
                         start=(i == 0), stop=(i == nblk - 1))


def emit_ep(g, j, store=False):
    nc = g.nc
    osb = g.outts.tile([65, 512], BF16, tag="osb", name=f"osb{j}")
    nc.vector.tensor_copy(osb, g.avs[j])  # f32 PSUM -> bf16 SBUF
    for t in range(4):
        op = g.ps_mix.tile([128, H + 1], F32, tag="mix", name=f"op{j}_{t}")
        nc.tensor.matmul(op, lhsT=osb[:, t * 128:(t + 1) * 128],
                         rhs=g.i65_sb, start=True, stop=True)
        rc = g.smalls.tile([128, 1], F32, tag="rc", name=f"rc{j}_{t}")
        nc.vector.reciprocal(rc, op[:, H:H + 1])
        tb = j * 4 + t
        nc.vector.tensor_scalar_mul(g.outbuf[:, tb, :], op[:, 0:H], rc)
        if store:  # last slice: store per block for a short tail
            nc.sync.dma_start(out=g.out_d[tb * 128:(tb + 1) * 128, :],
                              in_=g.outbuf[:, tb, :])


_CACHED = {}


def _get_nc():
    if "nc" not in _CACHED:
        from contextlib import ExitStack
        nc = bacc.Bacc("TRN2", target_bir_lowering=False, debug=False,
                       num_devices=B)
        with tile.TileContext(nc) as tc:
            with ExitStack() as ctx:
                build_attention(nc, tc, ctx)
        nc.compile()
        _CACHED["nc"] = nc
    return _CACHED["nc"]


def _quant_inputs(inputs, Wq, Wk, Wv):
    """Host-side prep: xT in [128, 8, T] bf16 layout, packed [Wq|Wk]."""
    inputs = np.asarray(inputs, dtype=np.float32)

    def wlayout(w, m):  # [C, m] -> [128, 8, m]
        return np.ascontiguousarray(
            np.asarray(w).astype(npbf16).reshape(8, 128, m).transpose(
                1, 0, 2))

    wqk = wlayout(np.concatenate([np.asarray(Wq), np.asarray(Wk)], axis=1),
                  128)
    wv = wlayout(Wv, H)

    idents = np.zeros((128, 257), dtype=npbf16)
    idents[0:64, 0:64] = np.eye(64, dtype=npbf16)
    idents[64:128, 0:64] = np.eye(64, dtype=npbf16)
    idents[:, 64:192] = np.triu(np.ones((128, 128), dtype=npbf16))
    idents[0:65, 192:257] = np.eye(65, dtype=npbf16)

    in_maps = []
    for b in range(B):
        xT = np.ascontiguousarray(
            inputs[b].T.astype(npbf16).reshape(8, 128, T).transpose(1, 0, 2))
        in_maps.append({"xT": xT, "wqk": wqk, "wv": wv, "idents": idents})
    return in_maps


def kernel(inputs, Wq, Wk, Wv):
    in_maps = _quant_inputs(inputs, Wq, Wk, Wv)
    nc = _get_nc()
    res = run_bass_kernel_spmd(nc, in_maps, core_ids=list(range(B)))
    out = np.stack([res.results[b]["out"] for b in range(B)], axis=0)
    return out.astype(np.float32)
